# revision 18
# baseline (speedup 1.0000x reference)
"""Trainium2 Bass kernel for nn_Attention_86732569575984 (B2 L2048 D1024 H16).

Sharding: 8 cores = 2 batches x 4 head-groups (4 heads/core).
Per-core (fp32): qkv^T via weights-stationary matmul ([e,t] layout), rope in
[d,t] (32-partition block swap via SBUF DMA), rmsnorm factors via
ones-matmul row sums + Ln/Exp, scores^T [j,i] on PE (bounded, no max-sub),
exp on ACT straight from PSUM, attn out via [v|1]-stationary matmul
(denominator ride-along), reciprocal_approx normalize, proj_w slice matmul.
Host: pre-transpose/reorder inputs; sum partial outs over 4 cores/batch.
"""
import math
import os
from contextlib import ExitStack
KPHASE = int(os.environ.get("KPHASE", "3"))

import numpy as np

import concourse.bass as bass
import concourse.mybir as mybir
import concourse.tile as tile
from concourse import bacc
from concourse.bass_utils import run_bass_kernel_spmd

B, L, DIM, H = 2, 2048, 1024, 16
HD = 64
HPC = 4
N_CORES = 8
EPS = 1e-6
RATIO = math.sqrt(math.log(L) / math.log(1040.0))
F32 = mybir.dt.float32
AF = mybir.ActivationFunctionType

NT = L // 128
VW = 260            # per-head 65 cols: [v(64) | ones]


def _bcast_rows(src_row, n):
    # [1, F] -> [1, n, F] with zero-stride repeat dim; pairs with an
    # [n, F] DMA destination in iteration order.
    ap = [list(d) for d in src_row.ap]
    return bass.AP(src_row.tensor, src_row.offset,
                   [ap[0], [0, n]] + ap[1:])


def _emit(nc, io):
    ctx = ExitStack()
    with tile.TileContext(nc) as tc, ctx:
        p = ctx.enter_context(tc.tile_pool(name="persist", bufs=1))
        rot = ctx.enter_context(tc.tile_pool(name="rot", bufs=2))

        cos4 = p.tile([128, L], F32)
        sin4m = p.tile([128, L], F32)
        nc.sync.dma_start(cos4[:], io["cos4"].ap()[:])
        nc.sync.dma_start(sin4m[:], io["sin4m"].ap()[:])
        qk = [p.tile([128, L], F32, name=f"qk{i}", tag=f"qk{i}")
              for i in range(4)]
        vt_sb = [p.tile([128, VW], F32, name=f"vsb{i}", tag=f"vsb{i}")
                 for i in range(NT)]
        outT = [p.tile([128, L], F32, name=f"oT{i}", tag=f"oT{i}")
                for i in range(2)]
        ssrow = p.tile([36, L], F32)   # q rows 0-3, k rows 32-35
        rr = p.tile([36, L], F32)
        den16 = p.tile([4, 4 * 512], F32)   # [ich, h*512]
        rec16 = p.tile([4, 4 * 512], F32)
        rscr = p.tile([4, 4 * 512], F32)
        ones_t = p.tile([128, 1], F32)
        nc.vector.memset(ones_t[:], 1.0)
        biasq = p.tile([4, 1], F32)
        nc.vector.memset(biasq[:], float(EPS))
        biask = p.tile([4, 1], F32)
        nc.vector.memset(biask[:], float(HD * EPS / (RATIO * RATIO)))
        zb = p.tile([128, 1], F32)
        nc.vector.memset(zb[:], 0.0)

        xTr = io["xT"].ap().rearrange("(a p) n -> p a n", p=128)
        v0ap = io["v0p"].ap()
        voap = io["vout"].ap()

        # ---------------- phase 1: qkv + rope + v blend + SS -------------
        ph1 = ExitStack()
        with ph1:
            wq = ph1.enter_context(tc.tile_pool(name="wq", bufs=1))
            xp = ph1.enter_context(tc.tile_pool(name="xp", bufs=2))
            ps1 = ph1.enter_context(
                tc.tile_pool(name="ps1", bufs=2, space="PSUM"))
            ps1v = ph1.enter_context(
                tc.tile_pool(name="ps1v", bufs=2, space="PSUM"))
            ps_ss = ph1.enter_context(
                tc.tile_pool(name="ps_ss", bufs=2, space="PSUM"))

            wqkT = wq.tile([128, 8 * 512], F32)
            nc.sync.dma_start(
                wqkT[:].rearrange("p (a n) -> p a n", n=512),
                io["wqkT"].ap().rearrange("(a p) n -> p a n", p=128))
            wvT = wq.tile([128, 8 * 256], F32)
            nc.sync.dma_start(
                wvT[:].rearrange("p (a n) -> p a n", n=256),
                io["wvT"].ap().rearrange("(a p) n -> p a n", p=128))

            for tch in range(8):
                xc = xp.tile([128, 8 * 256], F32, tag="xc", name=f"xc{tch}")
                nc.sync.dma_start(
                    xc[:].rearrange("p (a n) -> p a n", n=256),
                    xTr[:, :, tch * 256:(tch + 1) * 256])
                sl = slice(tch * 256, (tch + 1) * 256)
                for et in range(4):
                    pq = ps1.tile([128, 256], F32, tag="pq", name=f"pq{tch}{et}")
                    for d in range(8):
                        nc.tensor.matmul(
                            pq[:],
                            wqkT[:, d * 512 + et * 128:d * 512 + (et + 1) * 128],
                            xc[:, d * 256:(d + 1) * 256],
                            start=(d == 0), stop=(d == 7))
                    nc.vector.tensor_mul(qk[et][:, sl], pq[:], cos4[:, sl])
                    qr = rot.tile([128, 256], F32, tag="qraw", bufs=3,
                                  name=f"qr{tch}{et}")
                    nc.vector.tensor_copy(qr[:], pq[:])
                    xs = rot.tile([128, 256], F32, tag="xs", bufs=3,
                                  name=f"xs{tch}{et}")
                    for blk in range(4):
                        src = blk ^ 1
                        nc.gpsimd.dma_start(
                            xs[blk * 32:(blk + 1) * 32, :],
                            qr[src * 32:(src + 1) * 32, :])
                    nc.vector.tensor_mul(xs[:], xs[:], sin4m[:, sl])
                    nc.vector.tensor_add(qk[et][:, sl], qk[et][:, sl], xs[:])
                for vtl in range(2):
                    vt = tch * 2 + vtl
                    pv = ps1v.tile([128, 256], F32, tag="pv", name=f"pv{vt}")
                    for d in range(8):
                        nc.tensor.matmul(
                            pv[:],
                            xc[:, d * 256 + vtl * 128:d * 256 + (vtl + 1) * 128],
                            wvT[:, d * 256:(d + 1) * 256],
                            start=(d == 0), stop=(d == 7))
                    vs = vt_sb[vt]
                    vv = vs[:].rearrange("p (hh s) -> p hh s", s=65)
                    nc.gpsimd.memset(vv[:, :, 64:65], 1.0)
                    v0r = v0ap[vt * 128:(vt + 1) * 128, :]
                    nc.sync.dma_start(
                        vv[:, :, 0:64],
                        v0r.rearrange("p (a n) -> p a n", n=64))
                    pvv = pv[:].rearrange("p (h n) -> p h n", n=64)
                    nc.vector.tensor_add(vv[:, :, 0:64], pvv[:],
                                         vv[:, :, 0:64])
                    nc.gpsimd.dma_start(
                        voap[vt * 128:(vt + 1) * 128, :].rearrange(
                            "p (h n) -> p h n", n=64),
                        vv[:, :, 0:64])

            # squares + SS row sums
            for et in range(4):
                for half in range(2):
                    pss = [ps_ss.tile([1, 1024], F32, tag=f"pss{hh}", bufs=1,
                                      name=f"pss{et}{half}{hh}")
                           for hh in range(2)]
                    for j in range(2):
                        sch = half * 2 + j
                        sqc = rot.tile([128, 512], F32, tag="sq", bufs=3,
                                       name=f"sq{et}{sch}")
                        nc.scalar.square(
                            sqc[:], qk[et][:, sch * 512:(sch + 1) * 512])
                        for hh in range(2):
                            nc.tensor.matmul(
                                pss[hh][:, j * 512:(j + 1) * 512],
                                ones_t[hh * 64:hh * 64 + 64, :],
                                sqc[hh * 64:hh * 64 + 64, :],
                                start=True, stop=True)
                    for hh in range(2):
                        rowt = rot.tile([1, 1024], F32, tag="ssr", bufs=2,
                                        name=f"ssr{et}{half}{hh}")
                        nc.vector.tensor_copy(rowt[:], pss[hh][:])
                        r = (et * 2 + hh) if et < 2 else \
                            (32 + (et - 2) * 2 + hh)
                        nc.sync.dma_start(
                            ssrow[r:r + 1,
                                  half * 1024:(half + 1) * 1024],
                            rowt[:])

        # rr = exp(-0.5 * ln(scale*SS + bias)) ; rows 0-3 q, 4-7 k(w/ c/8)
        nc.scalar.activation(rr[0:4, :], ssrow[0:4, :], AF.Ln,
                             bias=biasq[:], scale=1.0 / HD)
        nc.scalar.activation(rr[32:36, :], ssrow[32:36, :], AF.Ln,
                             bias=biask[:], scale=1.0)
        nc.scalar.activation(rr[0:4, :], rr[0:4, :], AF.Exp, scale=-0.5,
                             bias=zb[0:4, :])
        nc.scalar.activation(rr[32:36, :], rr[32:36, :], AF.Exp, scale=-0.5,
                             bias=zb[0:4, :])

        for et in range(4):
            for ich in range(4):
                rqb = rot.tile([128, 512], F32, tag="rqb", bufs=3,
                               name=f"rqb{et}{ich}")
                for hh in range(2):
                    r = (et * 2 + hh) if et < 2 else \
                        (32 + (et - 2) * 2 + hh)
                    src = rr[r:r + 1, ich * 512:(ich + 1) * 512]
                    nc.sync.dma_start(rqb[hh * 64:(hh + 1) * 64, :],
                                      _bcast_rows(src, 64))
                nc.vector.tensor_mul(
                    qk[et][:, ich * 512:(ich + 1) * 512],
                    qk[et][:, ich * 512:(ich + 1) * 512], rqb[:])

        if KPHASE < 2:
            ob0 = rot.tile([128, 512], F32, tag="ob0")
            nc.vector.memset(ob0[:], 0.0)
            for ec in range(8):
                for ich in range(4):
                    nc.sync.dma_start(
                        io["outp"].ap()[ec * 128:(ec + 1) * 128,
                                        ich * 512:(ich + 1) * 512], ob0[:])
            return
        # ---------------- phase 2: attention -----------------------------
        ph2 = ExitStack()
        with ph2:
            ps_s = ph2.enter_context(
                tc.tile_pool(name="ps_s", bufs=1, space="PSUM"))
            ps_o = ph2.enter_context(
                tc.tile_pool(name="ps_o", bufs=4, space="PSUM"))
            dsb = ph2.enter_context(tc.tile_pool(name="dsb", bufs=4))
            exp = ph2.enter_context(tc.tile_pool(name="exp", bufs=2))
            for h in range(HPC):
                et, hh = h // 2, h % 2
                qn, kn = qk[et], qk[2 + et]
                po = [ps_o.tile([128, 512], F32, tag=f"po{i}", bufs=1,
                                name=f"po{h}_{i}") for i in range(4)]
                for jt in range(NT):
                    pscr = ps_s.tile([128, L], F32, tag="pscr",
                                     name=f"sc{h}_{jt}")
                    for ich in range(4):
                        nc.tensor.matmul(
                            pscr[:, ich * 512:(ich + 1) * 512],
                            kn[hh * 64:hh * 64 + 64, jt * 128:(jt + 1) * 128],
                            qn[hh * 64:hh * 64 + 64,
                               ich * 512:(ich + 1) * 512],
                            start=True, stop=True)
                    ex = exp.tile([128, L], F32, tag="ex", name=f"ex{h}_{jt}")
                    nc.scalar.activation(ex[:], pscr[:], AF.Exp)
                    lhs = vt_sb[jt][:, h * 65:h * 65 + 65]
                    for ich in range(4):
                        nc.tensor.matmul(
                            po[ich][0:65, :], lhs,
                            ex[:, ich * 512:(ich + 1) * 512],
                            start=(jt == 0), stop=(jt == NT - 1))
                dr = 64
                for ich in range(4):
                    drow = dsb.tile([128, 512], F32, tag="drow",
                                    name=f"dr{h}_{ich}")
                    nc.vector.tensor_copy(drow[dr:dr + 1, :],
                                          po[ich][dr:dr + 1, :])
                    nc.sync.dma_start(
                        den16[ich:ich + 1, h * 512:(h + 1) * 512],
                        drow[dr:dr + 1, :])
                nc.vector.reciprocal_approx_accurate(
                    rec16[:, h * 512:(h + 1) * 512],
                    den16[:, h * 512:(h + 1) * 512],
                    rscr[:, h * 512:(h + 1) * 512])
                for ich in range(4):
                    rb = dsb.tile([128, 512], F32, tag="rb",
                                  name=f"rb{h}_{ich}")
                    src = rec16[ich:ich + 1, h * 512:(h + 1) * 512]
                    nc.sync.dma_start(rb[0:64, :], _bcast_rows(src, 64))
                    osl = slice(ich * 512, (ich + 1) * 512)
                    if hh == 0:
                        nc.vector.tensor_mul(outT[et][0:64, osl],
                                             po[ich][0:64, :], rb[0:64, :])
                    else:
                        tmp = dsb.tile([128, 512], F32, tag="otmp",
                                       name=f"ot{h}_{ich}")
                        nc.vector.tensor_mul(tmp[0:64, :], po[ich][0:64, :],
                                             rb[0:64, :])
                        nc.gpsimd.dma_start(outT[et][64:128, osl],
                                            tmp[0:64, :])

        if KPHASE < 3:
            ob0 = rot.tile([128, 512], F32, tag="ob0")
            nc.vector.memset(ob0[:], 0.0)
            for ec in range(8):
                for ich in range(4):
                    nc.sync.dma_start(
                        io["outp"].ap()[ec * 128:(ec + 1) * 128,
                                        ich * 512:(ich + 1) * 512], ob0[:])
            return
        # ---------------- phase 3: projection ----------------------------
        ph3 = ExitStack()
        with ph3:
            pwp = ph3.enter_context(tc.tile_pool(name="pwp", bufs=1))
            ps_p = ph3.enter_context(
                tc.tile_pool(name="ps_p", bufs=4, space="PSUM"))
            osb = ph3.enter_context(tc.tile_pool(name="osb", bufs=4))
            projw = pwp.tile([128, 2 * DIM], F32)
            nc.sync.dma_start(
                projw[:].rearrange("p (a n) -> p a n", n=DIM),
                io["pwT"].ap().rearrange("(a p) n -> p a n", p=128))
            for ec in range(8):
                for ich in range(4):
                    pp = ps_p.tile([128, 512], F32, tag="pp",
                                   name=f"pp{ec}_{ich}")
                    for dt in range(2):
                        nc.tensor.matmul(
                            pp[:],
                            projw[:, dt * DIM + ec * 128:
                                  dt * DIM + (ec + 1) * 128],
                            outT[dt][:, ich * 512:(ich + 1) * 512],
                            start=(dt == 0), stop=(dt == 1))
                    ob = osb.tile([128, 512], F32, tag="ob",
                                  name=f"ob{ec}_{ich}")
                    nc.vector.tensor_copy(ob[:], pp[:])
                    nc.sync.dma_start(
                        io["outp"].ap()[ec * 128:(ec + 1) * 128,
                                        ich * 512:(ich + 1) * 512], ob[:])


_CACHE = {}


def _build():
    if "nc" in _CACHE:
        return _CACHE["nc"]
    nc = bacc.Bacc("TRN2", target_bir_lowering=False, debug=False,
                   num_devices=N_CORES)
    io = {
        "xT": nc.dram_tensor("xT", [DIM, L], F32, kind="ExternalInput"),
        "wqkT": nc.dram_tensor("wqkT", [DIM, 512], F32, kind="ExternalInput"),
        "wvT": nc.dram_tensor("wvT", [DIM, 256], F32, kind="ExternalInput"),
        "v0p": nc.dram_tensor("v0p", [L, 256], F32, kind="ExternalInput"),
        "cos4": nc.dram_tensor("cos4", [128, L], F32, kind="ExternalInput"),
        "sin4m": nc.dram_tensor("sin4m", [128, L], F32, kind="ExternalInput"),
        "pwT": nc.dram_tensor("pwT", [256, DIM], F32, kind="ExternalInput"),
        "outp": nc.dram_tensor("outp", [DIM, L], F32, kind="ExternalOutput"),
        "vout": nc.dram_tensor("vout", [L, 256], F32, kind="ExternalOutput"),
    }
    _emit(nc, io)
    nc.compile()
    _CACHE["nc"] = nc
    return nc


def _prep_core(c, x, v_0, cos4, sin4m, wr, pw, lam):
    b, hg = c // 4, c % 4
    heads = [hg * 4 + j for j in range(HPC)]
    xT = np.ascontiguousarray(x[b].T)
    wq = wr[0, heads].reshape(HPC * HD, DIM)
    wk = wr[1, heads].reshape(HPC * HD, DIM)
    wqkT = np.ascontiguousarray(np.concatenate([wq, wk], 0).T)
    wvT = np.ascontiguousarray((lam * wr[2, heads].reshape(HPC * HD, DIM)).T)
    v0 = (1.0 - lam) * v_0[b][heads]          # [4, 2048, 64]
    v0p = np.ascontiguousarray(v0.transpose(1, 0, 2).reshape(L, HPC * HD))
    dims = np.concatenate([np.arange(h * HD, (h + 1) * HD) for h in heads])
    pwT = np.ascontiguousarray(pw[:, dims].T)
    return {"xT": xT, "wqkT": wqkT, "wvT": wvT, "v0p": v0p,
            "cos4": cos4, "sin4m": sin4m, "pwT": pwT}


def _prep_args(inputs):
    x = np.asarray(inputs["x"], np.float32)
    v_0 = np.asarray(inputs["v_0"], np.float32)
    lam = float(np.asarray(inputs["lambda_param"]).reshape(-1)[0])
    wr = np.asarray(inputs["qkv_w"], np.float32).reshape(3, H, HD, DIM)
    pw = np.asarray(inputs["proj_w"], np.float32)
    cT = np.asarray(inputs["rope_cos"], np.float32)[0, 0].T
    sT = np.asarray(inputs["rope_sin"], np.float32)[0, 0].T
    cos4 = np.ascontiguousarray(np.tile(cT, (4, 1)))
    sin4m = np.ascontiguousarray(np.concatenate([sT, -sT, sT, -sT], 0))
    return x, v_0, cos4, sin4m, wr, pw, lam


def run_cores(inputs, **kw):
    x = np.asarray(inputs["x"], np.float32)
    v_0 = np.asarray(inputs["v_0"], np.float32)
    lam = float(np.asarray(inputs["lambda_param"]).reshape(-1)[0])
    wr = np.asarray(inputs["qkv_w"], np.float32).reshape(3, H, HD, DIM)
    pw = np.asarray(inputs["proj_w"], np.float32)
    cT = np.asarray(inputs["rope_cos"], np.float32)[0, 0].T
    sT = np.asarray(inputs["rope_sin"], np.float32)[0, 0].T
    cos4 = np.ascontiguousarray(np.tile(cT, (4, 1)))
    sin4m = np.ascontiguousarray(np.concatenate([sT, -sT, sT, -sT], 0))
    nc = _build()
    in_maps = [_prep_core(c, x, v_0, cos4, sin4m, wr, pw, lam)
               for c in range(N_CORES)]
    return nc, run_bass_kernel_spmd(nc, in_maps,
                                    core_ids=list(range(N_CORES)), **kw)


def kernel(x, v_0, rope_cos, rope_sin, qkv_w, proj_w, lambda_param):
    _, res = run_cores(dict(x=x, v_0=v_0, rope_cos=rope_cos,
                            rope_sin=rope_sin, qkv_w=qkv_w, proj_w=proj_w,
                            lambda_param=lambda_param))
    out = np.zeros((B, L, DIM), np.float32)
    v = np.zeros((B, H, L, HD), np.float32)
    for c in range(N_CORES):
        b, hg = c // 4, c % 4
        out[b] += res.results[c]["outp"].T
        vc = res.results[c]["vout"].reshape(L, HPC, HD)
        for j in range(HPC):
            v[b, hg * 4 + j] = vc[:, j, :]
    return out, v


# revision 19
# speedup vs baseline: 81.8616x; 81.8616x over previous
"""Trainium2 Bass kernel for nn_Attention_86732569575984 (B2 L2048 D1024 H16).

Sharding: 8 cores = 2 batches x 4 head-groups (4 heads/core).
Per-core (fp32): qkv^T via weights-stationary matmul ([e,t] layout), rope in
[d,t] (32-partition block swap via SBUF DMA), rmsnorm factors via
ones-matmul row sums + Ln/Exp, scores^T [j,i] on PE (bounded, no max-sub),
exp on ACT straight from PSUM, attn out via [v|1]-stationary matmul
(denominator ride-along), reciprocal_approx normalize, proj_w slice matmul.
Host: pre-transpose/reorder inputs; sum partial outs over 4 cores/batch.
"""
import math
import os
from contextlib import ExitStack
KPHASE = int(os.environ.get("KPHASE", "3"))

import numpy as np

import concourse.bass as bass
import concourse.mybir as mybir
import concourse.tile as tile
from concourse import bacc
from concourse.bass_utils import run_bass_kernel_spmd

B, L, DIM, H = 2, 2048, 1024, 16
HD = 64
HPC = 4
N_CORES = 8
EPS = 1e-6
RATIO = math.sqrt(math.log(L) / math.log(1040.0))
F32 = mybir.dt.float32
AF = mybir.ActivationFunctionType

NT = L // 128
VW = 260            # per-head 65 cols: [v(64) | ones]


def _bcast_rows(src_row, n):
    # [1, F] -> [1, n, F] with zero-stride repeat dim; pairs with an
    # [n, F] DMA destination in iteration order.
    ap = [list(d) for d in src_row.ap]
    return bass.AP(src_row.tensor, src_row.offset,
                   [ap[0], [0, n]] + ap[1:])


def _emit(nc, io, reps=1):
    with tile.TileContext(nc) as tc:
        for _rep in range(reps):
            _emit_rep(nc, tc, io)


def _emit_rep(nc, tc, io):
    ctx = ExitStack()
    with ctx:
        p = ctx.enter_context(tc.tile_pool(name="persist", bufs=1))
        rot = ctx.enter_context(tc.tile_pool(name="rot", bufs=2))

        cos4 = p.tile([128, L], F32)
        sin4m = p.tile([128, L], F32)
        nc.sync.dma_start(cos4[:], io["cos4"].ap()[:])
        nc.sync.dma_start(sin4m[:], io["sin4m"].ap()[:])
        qk = [p.tile([128, L], F32, name=f"qk{i}", tag=f"qk{i}")
              for i in range(4)]
        vt_sb = [p.tile([128, VW], F32, name=f"vsb{i}", tag=f"vsb{i}")
                 for i in range(NT)]
        outT = [p.tile([128, L], F32, name=f"oT{i}", tag=f"oT{i}")
                for i in range(2)]
        ssrow = p.tile([36, L], F32)   # q rows 0-3, k rows 32-35
        rr = p.tile([36, L], F32)
        den16 = p.tile([4, 4 * 512], F32)   # [ich, h*512]
        rec16 = p.tile([4, 4 * 512], F32)
        rscr = p.tile([4, 4 * 512], F32)
        ones_t = p.tile([128, 1], F32)
        nc.vector.memset(ones_t[:], 1.0)
        biasq = p.tile([4, 1], F32)
        nc.vector.memset(biasq[:], float(EPS))
        biask = p.tile([4, 1], F32)
        nc.vector.memset(biask[:], float(HD * EPS / (RATIO * RATIO)))
        zb = p.tile([128, 1], F32)
        nc.vector.memset(zb[:], 0.0)

        xTr = io["xT"].ap().rearrange("(a p) n -> p a n", p=128)
        v0ap = io["v0p"].ap()
        voap = io["vout"].ap()

        # ---------------- phase 1: qkv + rope + v blend + SS -------------
        ph1 = ExitStack()
        with ph1:
            wq = ph1.enter_context(tc.tile_pool(name="wq", bufs=1))
            xp = ph1.enter_context(tc.tile_pool(name="xp", bufs=2))
            ps1 = ph1.enter_context(
                tc.tile_pool(name="ps1", bufs=2, space="PSUM"))
            ps1v = ph1.enter_context(
                tc.tile_pool(name="ps1v", bufs=2, space="PSUM"))
            ps_ss = ph1.enter_context(
                tc.tile_pool(name="ps_ss", bufs=2, space="PSUM"))

            wqkT = wq.tile([128, 8 * 512], F32)
            nc.sync.dma_start(
                wqkT[:].rearrange("p (a n) -> p a n", n=512),
                io["wqkT"].ap().rearrange("(a p) n -> p a n", p=128))
            wvT = wq.tile([128, 8 * 256], F32)
            nc.sync.dma_start(
                wvT[:].rearrange("p (a n) -> p a n", n=256),
                io["wvT"].ap().rearrange("(a p) n -> p a n", p=128))

            for tch in range(8):
                xc = xp.tile([128, 8 * 256], F32, tag="xc", name=f"xc{tch}")
                nc.sync.dma_start(
                    xc[:].rearrange("p (a n) -> p a n", n=256),
                    xTr[:, :, tch * 256:(tch + 1) * 256])
                sl = slice(tch * 256, (tch + 1) * 256)
                for et in range(4):
                    pq = ps1.tile([128, 256], F32, tag="pq", name=f"pq{tch}{et}")
                    for d in range(8):
                        nc.tensor.matmul(
                            pq[:],
                            wqkT[:, d * 512 + et * 128:d * 512 + (et + 1) * 128],
                            xc[:, d * 256:(d + 1) * 256],
                            start=(d == 0), stop=(d == 7))
                    nc.vector.tensor_mul(qk[et][:, sl], pq[:], cos4[:, sl])
                    qr = rot.tile([128, 256], F32, tag="qraw", bufs=3,
                                  name=f"qr{tch}{et}")
                    nc.vector.tensor_copy(qr[:], pq[:])
                    xs = rot.tile([128, 256], F32, tag="xs", bufs=3,
                                  name=f"xs{tch}{et}")
                    for blk in range(4):
                        src = blk ^ 1
                        nc.gpsimd.dma_start(
                            xs[blk * 32:(blk + 1) * 32, :],
                            qr[src * 32:(src + 1) * 32, :])
                    nc.vector.tensor_mul(xs[:], xs[:], sin4m[:, sl])
                    nc.vector.tensor_add(qk[et][:, sl], qk[et][:, sl], xs[:])
                for vtl in range(2):
                    vt = tch * 2 + vtl
                    pv = ps1v.tile([128, 256], F32, tag="pv", name=f"pv{vt}")
                    for d in range(8):
                        nc.tensor.matmul(
                            pv[:],
                            xc[:, d * 256 + vtl * 128:d * 256 + (vtl + 1) * 128],
                            wvT[:, d * 256:(d + 1) * 256],
                            start=(d == 0), stop=(d == 7))
                    vs = vt_sb[vt]
                    vv = vs[:].rearrange("p (hh s) -> p hh s", s=65)
                    nc.gpsimd.memset(vv[:, :, 64:65], 1.0)
                    v0r = v0ap[vt * 128:(vt + 1) * 128, :]
                    nc.sync.dma_start(
                        vv[:, :, 0:64],
                        v0r.rearrange("p (a n) -> p a n", n=64))
                    pvv = pv[:].rearrange("p (h n) -> p h n", n=64)
                    nc.vector.tensor_add(vv[:, :, 0:64], pvv[:],
                                         vv[:, :, 0:64])
                    nc.gpsimd.dma_start(
                        voap[vt * 128:(vt + 1) * 128, :].rearrange(
                            "p (h n) -> p h n", n=64),
                        vv[:, :, 0:64])

            # squares + SS row sums
            for et in range(4):
                for half in range(2):
                    pss = [ps_ss.tile([1, 1024], F32, tag=f"pss{hh}", bufs=1,
                                      name=f"pss{et}{half}{hh}")
                           for hh in range(2)]
                    for j in range(2):
                        sch = half * 2 + j
                        sqc = rot.tile([128, 512], F32, tag="sq", bufs=3,
                                       name=f"sq{et}{sch}")
                        nc.scalar.square(
                            sqc[:], qk[et][:, sch * 512:(sch + 1) * 512])
                        for hh in range(2):
                            nc.tensor.matmul(
                                pss[hh][:, j * 512:(j + 1) * 512],
                                ones_t[hh * 64:hh * 64 + 64, :],
                                sqc[hh * 64:hh * 64 + 64, :],
                                start=True, stop=True)
                    for hh in range(2):
                        rowt = rot.tile([1, 1024], F32, tag="ssr", bufs=2,
                                        name=f"ssr{et}{half}{hh}")
                        nc.vector.tensor_copy(rowt[:], pss[hh][:])
                        r = (et * 2 + hh) if et < 2 else \
                            (32 + (et - 2) * 2 + hh)
                        nc.sync.dma_start(
                            ssrow[r:r + 1,
                                  half * 1024:(half + 1) * 1024],
                            rowt[:])

        # rr = exp(-0.5 * ln(scale*SS + bias)) ; rows 0-3 q, 4-7 k(w/ c/8)
        nc.scalar.activation(rr[0:4, :], ssrow[0:4, :], AF.Ln,
                             bias=biasq[:], scale=1.0 / HD)
        nc.scalar.activation(rr[32:36, :], ssrow[32:36, :], AF.Ln,
                             bias=biask[:], scale=1.0)
        nc.scalar.activation(rr[0:4, :], rr[0:4, :], AF.Exp, scale=-0.5,
                             bias=zb[0:4, :])
        nc.scalar.activation(rr[32:36, :], rr[32:36, :], AF.Exp, scale=-0.5,
                             bias=zb[0:4, :])

        for et in range(4):
            for ich in range(4):
                rqb = rot.tile([128, 512], F32, tag="rqb", bufs=3,
                               name=f"rqb{et}{ich}")
                for hh in range(2):
                    r = (et * 2 + hh) if et < 2 else \
                        (32 + (et - 2) * 2 + hh)
                    src = rr[r:r + 1, ich * 512:(ich + 1) * 512]
                    nc.sync.dma_start(rqb[hh * 64:(hh + 1) * 64, :],
                                      _bcast_rows(src, 64))
                nc.vector.tensor_mul(
                    qk[et][:, ich * 512:(ich + 1) * 512],
                    qk[et][:, ich * 512:(ich + 1) * 512], rqb[:])

        if KPHASE < 2:
            ob0 = rot.tile([128, 512], F32, tag="ob0")
            nc.vector.memset(ob0[:], 0.0)
            for ec in range(8):
                for ich in range(4):
                    nc.sync.dma_start(
                        io["outp"].ap()[ec * 128:(ec + 1) * 128,
                                        ich * 512:(ich + 1) * 512], ob0[:])
            return
        # ---------------- phase 2: attention -----------------------------
        ph2 = ExitStack()
        with ph2:
            ps_s = ph2.enter_context(
                tc.tile_pool(name="ps_s", bufs=1, space="PSUM"))
            ps_o = ph2.enter_context(
                tc.tile_pool(name="ps_o", bufs=4, space="PSUM"))
            dsb = ph2.enter_context(tc.tile_pool(name="dsb", bufs=4))
            exp = ph2.enter_context(tc.tile_pool(name="exp", bufs=2))
            for h in range(HPC):
                et, hh = h // 2, h % 2
                qn, kn = qk[et], qk[2 + et]
                po = [ps_o.tile([128, 512], F32, tag=f"po{i}", bufs=1,
                                name=f"po{h}_{i}") for i in range(4)]
                for jt in range(NT):
                    pscr = ps_s.tile([128, L], F32, tag="pscr",
                                     name=f"sc{h}_{jt}")
                    for ich in range(4):
                        nc.tensor.matmul(
                            pscr[:, ich * 512:(ich + 1) * 512],
                            kn[hh * 64:hh * 64 + 64, jt * 128:(jt + 1) * 128],
                            qn[hh * 64:hh * 64 + 64,
                               ich * 512:(ich + 1) * 512],
                            start=True, stop=True)
                    ex = exp.tile([128, L], F32, tag="ex", name=f"ex{h}_{jt}")
                    nc.scalar.activation(ex[:], pscr[:], AF.Exp)
                    lhs = vt_sb[jt][:, h * 65:h * 65 + 65]
                    for ich in range(4):
                        nc.tensor.matmul(
                            po[ich][0:65, :], lhs,
                            ex[:, ich * 512:(ich + 1) * 512],
                            start=(jt == 0), stop=(jt == NT - 1))
                dr = 64
                for ich in range(4):
                    drow = dsb.tile([128, 512], F32, tag="drow",
                                    name=f"dr{h}_{ich}")
                    nc.vector.tensor_copy(drow[dr:dr + 1, :],
                                          po[ich][dr:dr + 1, :])
                    nc.sync.dma_start(
                        den16[ich:ich + 1, h * 512:(h + 1) * 512],
                        drow[dr:dr + 1, :])
                nc.vector.reciprocal_approx_accurate(
                    rec16[:, h * 512:(h + 1) * 512],
                    den16[:, h * 512:(h + 1) * 512],
                    rscr[:, h * 512:(h + 1) * 512])
                for ich in range(4):
                    rb = dsb.tile([128, 512], F32, tag="rb",
                                  name=f"rb{h}_{ich}")
                    src = rec16[ich:ich + 1, h * 512:(h + 1) * 512]
                    nc.sync.dma_start(rb[0:64, :], _bcast_rows(src, 64))
                    osl = slice(ich * 512, (ich + 1) * 512)
                    if hh == 0:
                        nc.vector.tensor_mul(outT[et][0:64, osl],
                                             po[ich][0:64, :], rb[0:64, :])
                    else:
                        tmp = dsb.tile([128, 512], F32, tag="otmp",
                                       name=f"ot{h}_{ich}")
                        nc.vector.tensor_mul(tmp[0:64, :], po[ich][0:64, :],
                                             rb[0:64, :])
                        nc.gpsimd.dma_start(outT[et][64:128, osl],
                                            tmp[0:64, :])

        if KPHASE < 3:
            ob0 = rot.tile([128, 512], F32, tag="ob0")
            nc.vector.memset(ob0[:], 0.0)
            for ec in range(8):
                for ich in range(4):
                    nc.sync.dma_start(
                        io["outp"].ap()[ec * 128:(ec + 1) * 128,
                                        ich * 512:(ich + 1) * 512], ob0[:])
            return
        # ---------------- phase 3: projection ----------------------------
        ph3 = ExitStack()
        with ph3:
            pwp = ph3.enter_context(tc.tile_pool(name="pwp", bufs=1))
            ps_p = ph3.enter_context(
                tc.tile_pool(name="ps_p", bufs=4, space="PSUM"))
            osb = ph3.enter_context(tc.tile_pool(name="osb", bufs=4))
            projw = pwp.tile([128, 2 * DIM], F32)
            nc.sync.dma_start(
                projw[:].rearrange("p (a n) -> p a n", n=DIM),
                io["pwT"].ap().rearrange("(a p) n -> p a n", p=128))
            for ec in range(8):
                for ich in range(4):
                    pp = ps_p.tile([128, 512], F32, tag="pp",
                                   name=f"pp{ec}_{ich}")
                    for dt in range(2):
                        nc.tensor.matmul(
                            pp[:],
                            projw[:, dt * DIM + ec * 128:
                                  dt * DIM + (ec + 1) * 128],
                            outT[dt][:, ich * 512:(ich + 1) * 512],
                            start=(dt == 0), stop=(dt == 1))
                    ob = osb.tile([128, 512], F32, tag="ob",
                                  name=f"ob{ec}_{ich}")
                    nc.vector.tensor_copy(ob[:], pp[:])
                    nc.sync.dma_start(
                        io["outp"].ap()[ec * 128:(ec + 1) * 128,
                                        ich * 512:(ich + 1) * 512], ob[:])


_CACHE = {}


def _build(reps=1):
    key = ("nc", reps)
    if key in _CACHE:
        return _CACHE[key]
    nc = bacc.Bacc("TRN2", target_bir_lowering=False, debug=False,
                   num_devices=N_CORES)
    io = {
        "xT": nc.dram_tensor("xT", [DIM, L], F32, kind="ExternalInput"),
        "wqkT": nc.dram_tensor("wqkT", [DIM, 512], F32, kind="ExternalInput"),
        "wvT": nc.dram_tensor("wvT", [DIM, 256], F32, kind="ExternalInput"),
        "v0p": nc.dram_tensor("v0p", [L, 256], F32, kind="ExternalInput"),
        "cos4": nc.dram_tensor("cos4", [128, L], F32, kind="ExternalInput"),
        "sin4m": nc.dram_tensor("sin4m", [128, L], F32, kind="ExternalInput"),
        "pwT": nc.dram_tensor("pwT", [256, DIM], F32, kind="ExternalInput"),
        "outp": nc.dram_tensor("outp", [DIM, L], F32, kind="ExternalOutput"),
        "vout": nc.dram_tensor("vout", [L, 256], F32, kind="ExternalOutput"),
    }
    _emit(nc, io, reps=reps)
    nc.compile()
    _CACHE[key] = nc
    return nc


def _prep_core(c, x, v_0, cos4, sin4m, wr, pw, lam):
    b, hg = c // 4, c % 4
    heads = [hg * 4 + j for j in range(HPC)]
    xT = np.ascontiguousarray(x[b].T)
    wq = wr[0, heads].reshape(HPC * HD, DIM)
    wk = wr[1, heads].reshape(HPC * HD, DIM)
    wqkT = np.ascontiguousarray(np.concatenate([wq, wk], 0).T)
    wvT = np.ascontiguousarray((lam * wr[2, heads].reshape(HPC * HD, DIM)).T)
    v0 = (1.0 - lam) * v_0[b][heads]          # [4, 2048, 64]
    v0p = np.ascontiguousarray(v0.transpose(1, 0, 2).reshape(L, HPC * HD))
    dims = np.concatenate([np.arange(h * HD, (h + 1) * HD) for h in heads])
    pwT = np.ascontiguousarray(pw[:, dims].T)
    return {"xT": xT, "wqkT": wqkT, "wvT": wvT, "v0p": v0p,
            "cos4": cos4, "sin4m": sin4m, "pwT": pwT}


def _prep_args(inputs):
    x = np.asarray(inputs["x"], np.float32)
    v_0 = np.asarray(inputs["v_0"], np.float32)
    lam = float(np.asarray(inputs["lambda_param"]).reshape(-1)[0])
    wr = np.asarray(inputs["qkv_w"], np.float32).reshape(3, H, HD, DIM)
    pw = np.asarray(inputs["proj_w"], np.float32)
    cT = np.asarray(inputs["rope_cos"], np.float32)[0, 0].T
    sT = np.asarray(inputs["rope_sin"], np.float32)[0, 0].T
    cos4 = np.ascontiguousarray(np.tile(cT, (4, 1)))
    sin4m = np.ascontiguousarray(np.concatenate([sT, -sT, sT, -sT], 0))
    return x, v_0, cos4, sin4m, wr, pw, lam


def run_cores(inputs, **kw):
    x = np.asarray(inputs["x"], np.float32)
    v_0 = np.asarray(inputs["v_0"], np.float32)
    lam = float(np.asarray(inputs["lambda_param"]).reshape(-1)[0])
    wr = np.asarray(inputs["qkv_w"], np.float32).reshape(3, H, HD, DIM)
    pw = np.asarray(inputs["proj_w"], np.float32)
    cT = np.asarray(inputs["rope_cos"], np.float32)[0, 0].T
    sT = np.asarray(inputs["rope_sin"], np.float32)[0, 0].T
    cos4 = np.ascontiguousarray(np.tile(cT, (4, 1)))
    sin4m = np.ascontiguousarray(np.concatenate([sT, -sT, sT, -sT], 0))
    nc = _build()
    in_maps = [_prep_core(c, x, v_0, cos4, sin4m, wr, pw, lam)
               for c in range(N_CORES)]
    return nc, run_bass_kernel_spmd(nc, in_maps,
                                    core_ids=list(range(N_CORES)), **kw)


def kernel(x, v_0, rope_cos, rope_sin, qkv_w, proj_w, lambda_param):
    _, res = run_cores(dict(x=x, v_0=v_0, rope_cos=rope_cos,
                            rope_sin=rope_sin, qkv_w=qkv_w, proj_w=proj_w,
                            lambda_param=lambda_param))
    out = np.zeros((B, L, DIM), np.float32)
    v = np.zeros((B, H, L, HD), np.float32)
    for c in range(N_CORES):
        b, hg = c // 4, c % 4
        out[b] += res.results[c]["outp"].T
        vc = res.results[c]["vout"].reshape(L, HPC, HD)
        for j in range(HPC):
            v[b, hg * 4 + j] = vc[:, j, :]
    return out, v
